# revision 36
# baseline (speedup 1.0000x reference)
"""Trainium2 Bass kernel: per-token multi-head self-attention (fused, bf16).

Computation (per token t):
  q,k,v = x @ W{q,k,v}.T ; scores = (q_t k_t^T)/sqrt(128) over heads [16x16]
  out_t = softmax(scores) @ v_t ; y = out @ Wo.T

Sharding: data-parallel over 16384 tokens -> 8 cores x 2048 tokens.
Per core the 2048 tokens run in 4 chunks of 512, all in one fused pass:

  A(c):   q,k,v for chunk c, token-major [d, token, head] in SBUF; weight
          tiles streamed from DRAM (host pre-tiled, bf16); PSUM copy-out on
          the scalar engine (strided dst), keeping the DVE free.
  mid(c): per-token 16x16 head attention in 8-token groups. ONE
          [128d,128(t g)]x[128d,128(t h)] matmul yields all 8 tokens'
          score blocks on its diagonal (off-diagonal products are garbage);
          exp ACT over a 4-group PSUM bank; copy_predicated with a static
          block-diagonal mask builds bd (true 16-block-diagonal, garbage
          dropped); V-block PE transpose gives vg [(t,g), d | ones]; one AV
          matmul per group produces out + softmax normalizer z; batched
          reciprocal+broadcast-multiply normalizes; PE transpose back to
          [d, (t,h)]. Wo matmuls of chunk c-1 interleave as PE filler.

All matmul operands are bf16 (PSUM accumulation fp32); output y is fp32.
"""
import math
from contextlib import ExitStack

import numpy as np

NCORES = 8
E = 2048          # hidden
NH = 16           # heads
HD = 128          # head dim
TPC = 2048        # tokens per core
TC = 512          # tokens per chunk
P = 128
NE = E // P       # 16 contraction tiles
NO = E // P       # 16 output tiles
CH = TPC // TC    # 4 chunks
NSB = TC // 32    # 16 superbatches (32 tokens = 4 groups) per chunk
SC = 1.0 / math.sqrt(HD)

_cached = {}


def _build_program():
    import concourse.bass as bass
    import concourse.tile as tile
    from concourse import bacc, mybir
    from concourse.masks import make_identity

    f32 = mybir.dt.float32
    bf16 = mybir.dt.bfloat16

    nc = bacc.Bacc("TRN2", target_bir_lowering=False, debug=False)
    u8 = mybir.dt.uint8

    x_d = nc.dram_tensor("xt", [CH, P, NE, TC], bf16, kind="ExternalInput").ap()
    w3_d = nc.dram_tensor("w3", [NO, P, 3 * NE, P], bf16, kind="ExternalInput").ap()
    wo_d = nc.dram_tensor("wot", [NO, P, NH, P], bf16, kind="ExternalInput").ap()
    bdm_d = nc.dram_tensor("bdm", [P, P], u8, kind="ExternalInput").ap()
    yT_d = nc.dram_tensor("yT", [E, TPC], f32, kind="ExternalOutput").ap()

    with tile.TileContext(nc) as tc, ExitStack() as ctx:
        glob = ctx.enter_context(tc.tile_pool(name="glob", bufs=1))
        xp = ctx.enter_context(tc.tile_pool(name="xp", bufs=2))
        wp = ctx.enter_context(tc.tile_pool(name="wp", bufs=2))
        esp = ctx.enter_context(tc.tile_pool(name="esp", bufs=2))
        aop = ctx.enter_context(tc.tile_pool(name="aop", bufs=2))
        wop = ctx.enter_context(tc.tile_pool(name="wop", bufs=2))
        invp = ctx.enter_context(tc.tile_pool(name="invp", bufs=2))
        aoup = ctx.enter_context(tc.tile_pool(name="aoup", bufs=2))
        aonp = ctx.enter_context(tc.tile_pool(name="aonp", bufs=2))
        ystp = ctx.enter_context(tc.tile_pool(name="ystp", bufs=2))
        psA = ctx.enter_context(tc.tile_pool(name="psA", bufs=2, space="PSUM"))
        psS = ctx.enter_context(tc.tile_pool(name="psS", bufs=2, space="PSUM"))
        psM = ctx.enter_context(tc.tile_pool(name="psM", bufs=3, space="PSUM"))
        psY = ctx.enter_context(tc.tile_pool(name="psY", bufs=1, space="PSUM"))

        ident = glob.tile([P, P], bf16)
        make_identity(nc, ident)
        bdm = glob.tile([P, P], u8, tag="bdm")
        nc.sync.dma_start(out=bdm, in_=bdm_d)

        # Warmup burst: ~3.5us of dummy PE activity inside the initial DMA
        # shadow lifts the HAM clock gate to 8/8 before A(0)'s first real
        # matmul (cold MMs otherwise run at 1.2 GHz for the first ~3.4us).
        for i in range(30):
            wu = psM.tile([P, P], f32, tag="m", name="wu")
            nc.tensor.matmul(wu, ident, ident, start=True, stop=True)

        # chunk activations, token-major [d, token, head] so an 8-token
        # slice flattens to one contiguous 128-col matmul operand
        qc = glob.tile([P, TC, NH], bf16, tag="qc")
        kc = glob.tile([P, TC, NH], bf16, tag="kc")
        vc = glob.tile([P, TC, NH], bf16, tag="vc")

        # bd: one slot per superbatch (zero off-block-diagonal persists;
        # copy_predicated only ever writes the diagonal blocks)
        bd_slots = []
        for i in range(NSB):
            t = glob.tile([P, 4 * P], bf16, tag=f"bd{i}")
            nc.vector.memset(t, 0.0)
            bd_slots.append(t)
        # vg: [(t,g), d | ones] per 8-token group, 2 superbatches deep
        vg_slots = []
        for i in range(8):
            t = glob.tile([P, HD + 1], bf16, tag=f"vg{i}")
            nc.vector.memset(t[:, HD:HD + 1], 1.0)
            vg_slots.append(t)

        x_tiles = []

        def issue_x(c):
            xt = xp.tile([P, NE, TC], bf16, tag="xc", name="xt")
            nc.sync.dma_start(out=xt, in_=x_d[c])
            x_tiles.append(xt)

        # ---- A stage as a step machine: 32 q/k groups then 16 v groups,
        # so leading groups can interleave into the previous middle stage
        # (v last: its copies WAR against the previous chunk's V reads) ----
        def a_qk_prefetch(st):
            oi = st["qk_load"]
            if oi < NO:
                t = wp.tile([P, 2 * NE, P], bf16, tag="wqk", name="wqk")
                nc.sync.dma_start(out=t, in_=w3_d[oi][:, 0:2 * NE, :])
                st["qk_tiles"].append(t)
                st["qk_load"] += 1

        def a_v_prefetch(st):
            oi = st["v_load"]
            if oi < NO:
                t = wp.tile([P, NE, P], bf16, tag="wv", name="wv")
                nc.sync.dma_start(out=t, in_=w3_d[oi][:, 2 * NE:3 * NE, :])
                st["v_tiles"].append(t)
                st["v_load"] += 1

        def a_begin():
            st = {"xc": x_tiles.pop(0), "pos": 0,
                  "qk_tiles": [], "v_tiles": [], "qk_load": 0, "v_load": 0,
                  "wt": None}
            a_qk_prefetch(st)
            a_qk_prefetch(st)
            return st

        def a_step(st, nsteps, limit=48):
            for _ in range(nsteps):
                if st is None or st["pos"] >= limit:
                    return
                i = st["pos"]
                st["pos"] += 1
                if i < 32:
                    oi, m = i // 2, i % 2
                    if m == 0:
                        st["wt"] = st["qk_tiles"].pop(0)
                        a_qk_prefetch(st)
                    wsl = st["wt"][:, m * NE:(m + 1) * NE, :]
                    if i == 28 or i == 30:
                        a_v_prefetch(st)
                else:
                    oi, m = i - 32, 2
                    wsl = st["v_tiles"].pop(0)
                    a_v_prefetch(st)
                ps = psA.tile([P, TC], f32, tag="acc")
                for e in range(NE):
                    nc.tensor.matmul(
                        ps, wsl[:, e, :], st["xc"][:, e, :],
                        start=(e == 0), stop=(e == NE - 1))
                nc.scalar.activation(
                    out=dsts[m][:, :, oi], in_=ps,
                    func=mybir.ActivationFunctionType.Copy)

        # ---- Wo interleaved stream over the previous chunk ----
        wo_seq = [(oi, h) for oi in range(NO) for h in range(NH)]

        def wo_prefetch(state):
            oi = state["next_load"]
            if oi < NO:
                wt = wop.tile([P, NH, P], bf16, tag="wo", name="wo")
                nc.sync.dma_start(out=wt, in_=wo_d[oi])
                state["tiles"].append(wt)
                state["next_load"] += 1

        def wo_begin(aoT, t0):
            st = {"pos": 0, "aoT": aoT, "t0": t0, "tiles": [],
                  "next_load": 0, "wo": None, "yp": None}
            wo_prefetch(st)
            wo_prefetch(st)
            return st

        def wo_step(state, nsteps):
            for _ in range(nsteps):
                if state is None or state["pos"] >= len(wo_seq):
                    return
                oi, h = wo_seq[state["pos"]]
                state["pos"] += 1
                if h == 0:
                    state["wo"] = state["tiles"].pop(0)
                    wo_prefetch(state)
                    # alternate accumulator between psY and the psA banks
                    # (idle while the Wo stream runs) for 2-deep rotation
                    if oi % 2:
                        state["yp"] = psA.tile([P, TC], f32, tag="acc",
                                               name="yp_a")
                    else:
                        state["yp"] = psY.tile([P, TC], f32, tag="yp",
                                               name="yp")
                nc.tensor.matmul(
                    state["yp"], state["wo"][:, h, :], state["aoT"][:, h, :],
                    start=(h == 0), stop=(h == NH - 1))
                if h == NH - 1:
                    ys = ystp.tile([P, TC], f32, tag="ys")
                    nc.scalar.activation(
                        out=ys, in_=state["yp"],
                        func=mybir.ActivationFunctionType.Copy)
                    nc.sync.dma_start(
                        out=yT_d[oi * P:(oi + 1) * P,
                                 state["t0"]:state["t0"] + TC],
                        in_=ys)

        dsts = [qc, kc, vc]
        issue_x(0)
        ast = a_begin()
        prev = None
        for c in range(CH):
            # ================= A stage: q,k,v for chunk c =================
            with nc.named_scope(f"A{c}"):
                a_step(ast, 48)

            # ================= middle stage (+ Wo of chunk c-1) ============
            with nc.named_scope(f"M{c}"):
                if c + 1 < CH:
                    issue_x(c + 1)
                next_ast = a_begin() if c + 1 < CH else None
                aoT = aop.tile([P, NH, TC], bf16, tag="aoT")

                # all scores upfront so qc/kc are free for the next A stage
                for k in range(NSB):
                    sc = psS.tile([P, 4 * P], f32, tag="sc", name="sc")
                    for g in range(4):
                        tt = k * 32 + g * 8
                        nc.tensor.matmul(
                            sc[:, g * P:(g + 1) * P],
                            kc[:, tt:tt + 8, :]
                            .rearrange("p t h -> p (t h)"),
                            qc[:, tt:tt + 8, :]
                            .rearrange("p t h -> p (t h)"),
                            start=True, stop=True)
                    es = esp.tile([P, 4 * P], bf16, tag="es", name="es")
                    nc.scalar.activation(
                        out=es, in_=sc,
                        func=mybir.ActivationFunctionType.Exp, scale=SC)
                    bd = bd_slots[k]
                    for g in range(4):
                        nc.vector.copy_predicated(
                            bd[:, g * P:(g + 1) * P], bdm,
                            es[:, g * P:(g + 1) * P])
                    wo_step(prev, 2)

                def vgt_block(k):
                    for g in range(4):
                        tt = k * 32 + g * 8
                        vg_ps = psM.tile([P, P], bf16, tag="m", name="vg_ps")
                        nc.tensor.transpose(
                            vg_ps,
                            vc[:, tt:tt + 8, :]
                            .rearrange("p t h -> p (t h)"),
                            ident)
                        nc.vector.tensor_copy(
                            vg_slots[(k * 4 + g) % 8][:, 0:HD], vg_ps)

                vgt_block(0)
                for k in range(NSB):
                    if k + 1 < NSB:
                        vgt_block(k + 1)
                    bd = bd_slots[k]
                    ao_un = aoup.tile([P, 4, HD + 4], bf16, tag="aou",
                                      name="ao_un")
                    for g in range(4):
                        av = psM.tile([P, HD + 4], f32, tag="m", name="av")
                        nc.tensor.matmul(
                            av[:, 0:HD + 1],
                            bd[:, g * P:(g + 1) * P],
                            vg_slots[(k * 4 + g) % 8],
                            start=True, stop=True)
                        nc.vector.tensor_copy(
                            ao_un[:, g, 0:HD + 1], av[:, 0:HD + 1])
                        if prev is not None:
                            wo_step(prev, 2)
                        elif g % 2:
                            a_step(next_ast, 1, limit=32)
                    invz = invp.tile([P, 4], f32, tag="invz")
                    nc.vector.reciprocal(invz, ao_un[:, :, HD])
                    ao_n = aonp.tile([P, 4, HD], bf16, tag="aon",
                                     name="ao_n")
                    nc.vector.tensor_tensor(
                        ao_n, ao_un[:, :, 0:HD],
                        invz[:, :, None].to_broadcast((P, 4, HD)),
                        mybir.AluOpType.mult)
                    if prev is not None:
                        wo_step(prev, 2)
                    else:
                        a_step(next_ast, 1, limit=32)
                    for g in range(4):
                        tt = k * 32 + g * 8
                        aoT_ps = psM.tile([P, P], bf16, tag="m",
                                          name="aoT_ps")
                        nc.tensor.transpose(aoT_ps, ao_n[:, g, :], ident)
                        nc.vector.tensor_copy(
                            aoT[:, :, tt:tt + 8],
                            aoT_ps.rearrange("p (t h) -> p h t", t=8))
                        wo_step(prev, 1)
                wo_step(prev, len(wo_seq))
                prev = wo_begin(aoT, c * TC)
                ast = next_ast
        wo_step(prev, len(wo_seq))

    nc.compile()
    return nc


def _get_program():
    if "nc" not in _cached:
        _cached["nc"] = _build_program()
    return _cached["nc"]


def kernel(x, Wq, Wk, Wv, Wo):
    from concourse.bass_utils import run_bass_kernel_spmd
    import ml_dtypes

    bf = ml_dtypes.bfloat16
    B, S, H = x.shape
    assert (B * S, H) == (NCORES * TPC, E)
    nc = _get_program()

    xf = np.asarray(x, dtype=np.float32).reshape(B * S, H)

    def tile_w(WT):
        # WT [E(e-rows), E(f-cols)] -> [NO, P, NE, P] (per-oi contiguous)
        return np.ascontiguousarray(
            WT.reshape(NE, P, NO, P).transpose(2, 1, 0, 3)).astype(bf)

    w3 = np.ascontiguousarray(np.concatenate(
        [tile_w(Wq.T), tile_w(Wk.T), tile_w(Wv.T)], axis=2))
    wo_t = np.ascontiguousarray(
        Wo.T.reshape(NH, P, NO, P).transpose(2, 1, 0, 3)).astype(bf)
    bdm = np.kron(np.eye(8), np.ones((16, 16))).astype(np.uint8)

    in_maps = []
    for i in range(NCORES):
        xs = xf[i * TPC:(i + 1) * TPC, :].T  # [E, TPC]
        x_t = np.ascontiguousarray(
            xs.reshape(NE, P, CH, TC).transpose(2, 1, 0, 3)).astype(bf)
        in_maps.append({"xt": x_t, "w3": w3, "wot": wo_t, "bdm": bdm})

    import os
    trace = bool(int(os.environ.get("BASS_KERNEL_TRACE", "0")))
    res = run_bass_kernel_spmd(nc, in_maps, core_ids=list(range(NCORES)),
                               trace=trace)
    if trace:
        _cached["last_results"] = res
    parts = [res.results[i]["yT"].T for i in range(NCORES)]
    y = np.concatenate(parts, axis=0).reshape(B, S, H)
    return np.ascontiguousarray(y.astype(np.float32))
